# revision 3
# baseline (speedup 1.0000x reference)
"""Fused fake-quant GEMM + bias + residual + LayerNorm (BertSelfOutput) on 8 trn2 cores.

Strategy: data-parallel over the batch dim (B=8 -> one batch element per core).
Each core computes, for its [4096, 1024] shard:
    hq = fake_quant(hidden); wq = fake_quant(weight)
    h  = hq @ wq.T + bias;   y = h + input;   out = layernorm(y) * gamma + beta

Key trick: fake-quant values are integers in [-127, 127] after scaling, which
are exactly representable in bf16 -> the GEMM runs on the PE at full bf16 rate
with exact fp32 integer accumulation in PSUM; a single dequant multiply at the
end reproduces the reference to ~1e-6 relative.
"""

import numpy as np
import ml_dtypes

import concourse.bass as bass
import concourse.mybir as mybir
import concourse.tile as tile
from concourse import bacc
from concourse.bass_utils import run_bass_kernel_spmd

F32 = mybir.dt.float32
BF16 = mybir.dt.bfloat16
AF = mybir.ActivationFunctionType
OP = mybir.AluOpType

MAGIC = 12582912.0  # 1.5 * 2**23: (x + MAGIC) - MAGIC == rint(x) for |x| < 2**22
QMAX = 127.0
CLIP_VAL = 2.5
LN_EPS = 1e-12
H = 1024
N_CORES = 8
P = 128


def _scale_sym(x: np.ndarray) -> np.float32:
    """fp32-exact replica of the reference's per-tensor scale computation."""
    amax = np.float32(min(np.float32(np.abs(x).max()), np.float32(CLIP_VAL)))
    return np.float32(np.float32(QMAX) / np.maximum(amax, np.float32(1e-8)))


def build_bass(n_rows: int, s_h: float, s_w: float, deq: float, trivial_ln: bool):
    nc = bacc.Bacc(num_devices=N_CORES)
    KT = H // P  # 8 k-tiles
    MT = n_rows // P  # m-tiles per core

    hs = nc.declare_dram_parameter("hs", [n_rows, H], F32, isOutput=False)
    res = nc.declare_dram_parameter("res", [n_rows, H], F32, isOutput=False)
    w = nc.declare_dram_parameter("w", [H, H], F32, isOutput=False)
    biasq = nc.declare_dram_parameter("biasq", [2, H], BF16, isOutput=False)
    ones2 = nc.declare_dram_parameter("ones2", [2, P], BF16, isOutput=False)
    ident = nc.declare_dram_parameter("ident", [P, P], BF16, isOutput=False)
    if not trivial_ln:
        gamma = nc.declare_dram_parameter("gamma", [H], F32, isOutput=False)
        beta = nc.declare_dram_parameter("beta", [H], F32, isOutput=False)
    out = nc.declare_dram_parameter("out", [n_rows, H], F32, isOutput=True)

    with tile.TileContext(nc) as tc:
        with (
            tc.tile_pool(name="singles", bufs=1) as singles,
            tc.tile_pool(name="wprep", bufs=2) as wprep,
            tc.tile_pool(name="hin", bufs=3) as hin,
            tc.tile_pool(name="quant", bufs=2) as quant,
            tc.tile_pool(name="qt", bufs=2) as qtp,
            tc.tile_pool(name="resin", bufs=3) as resin,
            tc.tile_pool(name="yout", bufs=3) as yout,
            tc.tile_pool(name="stat", bufs=4) as stat,
            tc.tile_pool(name="pst", bufs=2, space="PSUM") as pst,
            tc.tile_pool(name="pso", bufs=2, space="PSUM") as pso_pool,
        ):
            # ---- constants
            id_t = singles.tile([P, P], BF16)
            nc.sync.dma_start(out=id_t, in_=ident[:, :])
            ones_t = singles.tile([2, P], BF16)
            nc.sync.dma_start(out=ones_t, in_=ones2[:, :])
            biasq_t = singles.tile([2, H], BF16)
            nc.sync.dma_start(out=biasq_t, in_=biasq[:, :])
            eps_t = singles.tile([P, 1], F32)
            nc.vector.memset(eps_t, LN_EPS)
            if not trivial_ln:
                gamma_t = singles.tile([P, H], F32)
                nc.sync.dma_start(
                    out=gamma_t,
                    in_=bass.AP(tensor=gamma.tensor, offset=0, ap=[[0, P], [1, H]]),
                )
                beta_t = singles.tile([P, H], F32)
                nc.sync.dma_start(
                    out=beta_t,
                    in_=bass.AP(tensor=beta.tensor, offset=0, ap=[[0, P], [1, H]]),
                )

            # ---- weight: quantize to integer-valued bf16, transpose to [h, o]
            # wqt[:, k, o] holds wq.T for k-tile k: partition = h within tile k.
            wqt = singles.tile([P, KT, H], BF16)
            for wb in range(KT):  # row-tiles of weight: [128 o, 1024 h]
                wt = wprep.tile([P, H], F32, tag="wt")
                nc.sync.dma_start(out=wt, in_=w[wb * P : (wb + 1) * P, :])
                uw = wprep.tile([P, H], F32, tag="uw")
                nc.scalar.activation(uw, wt, AF.Copy, bias=0.0, scale=float(s_w))
                rw = wprep.tile([P, H], BF16, tag="rw")
                nc.vector.tensor_scalar(
                    out=rw, in0=uw, scalar1=MAGIC, scalar2=MAGIC, op0=OP.add, op1=OP.subtract
                )
                qw = wprep.tile([P, H], BF16, tag="qw")
                nc.vector.tensor_scalar(
                    out=qw, in0=rw, scalar1=QMAX, scalar2=-QMAX, op0=OP.min, op1=OP.max
                )
                pt = pst.tile([P, H], F32, tag="pt")
                for k in range(KT):
                    # psum[:, k*128:+128] = qw[:, k*128:+128].T  ([128 h, 128 o])
                    nc.tensor.matmul(
                        pt[:, k * P : (k + 1) * P],
                        lhsT=qw[:, k * P : (k + 1) * P],
                        rhs=id_t[:, :],
                        start=True,
                        stop=True,
                    )
                # copy transposed blocks into wqt[:, :, wb*128:+128] (cast to bf16)
                nc.scalar.activation(
                    wqt[:, :, wb * P : (wb + 1) * P],
                    pt.rearrange("p (k o) -> p k o", k=KT),
                    AF.Copy,
                    bias=0.0,
                    scale=1.0,
                )

            # ---- main loop over m-tiles
            for m in range(MT):
                row = slice(m * P, (m + 1) * P)
                ht = hin.tile([P, H], F32)
                nc.sync.dma_start(out=ht, in_=hs[row, :])
                # u = hidden * s_h (fp32, single-rounded mult as in reference)
                ut = quant.tile([P, H], F32, tag="u")
                nc.scalar.activation(ut, ht, AF.Copy, bias=0.0, scale=float(s_h))
                # r = rint(u) -> integer-valued bf16 (exact for |r| <= 256)
                rt = quant.tile([P, H], BF16, tag="r")
                nc.vector.tensor_scalar(
                    out=rt, in0=ut, scalar1=MAGIC, scalar2=MAGIC, op0=OP.add, op1=OP.subtract
                )
                # q = clamp(r, -127, 127)
                qt = quant.tile([P, H], BF16, tag="q")
                nc.vector.tensor_scalar(
                    out=qt, in0=rt, scalar1=QMAX, scalar2=-QMAX, op0=OP.min, op1=OP.max
                )
                # transpose q into [h, m] layout via PE
                ptt = pst.tile([P, H], F32, tag="pt")
                for k in range(KT):
                    nc.tensor.matmul(
                        ptt[:, k * P : (k + 1) * P],
                        lhsT=qt[:, k * P : (k + 1) * P],
                        rhs=id_t[:, :],
                        start=True,
                        stop=True,
                    )
                qtt = qtp.tile([P, H], BF16)
                nc.scalar.activation(qtt, ptt, AF.Copy, bias=0.0, scale=1.0)

                # main GEMM: pso[m, o] = sum_k qtt[:, k].T @ wqt[:, k] (+ bias rows)
                pso = pso_pool.tile([P, H], F32)
                for nh in range(2):
                    col = slice(nh * 512, (nh + 1) * 512)
                    for k in range(KT):
                        nc.tensor.matmul(
                            pso[:, col],
                            lhsT=qtt[:, k * P : (k + 1) * P],
                            rhs=wqt[:, k, col],
                            start=(k == 0),
                            stop=False,
                        )
                    nc.tensor.matmul(
                        pso[:, col],
                        lhsT=ones_t[:, :],
                        rhs=biasq_t[:, col],
                        start=False,
                        stop=True,
                    )

                rt_in = resin.tile([P, H], F32)
                nc.sync.dma_start(out=rt_in, in_=res[row, :])
                # y = pso * deq + input  (dequant + residual fused)
                yt = yout.tile([P, H], F32, tag="y")
                nc.vector.scalar_tensor_tensor(
                    out=yt, in0=pso, scalar=float(deq), in1=rt_in, op0=OP.mult, op1=OP.add
                )
                # layernorm stats
                st = stat.tile([P, 2, 6], F32, tag="st")
                for c in range(2):
                    nc.vector.bn_stats(out=st[:, c, :], in_=yt[:, c * 512 : (c + 1) * 512])
                mv = stat.tile([P, 2], F32, tag="mv")
                nc.vector.bn_aggr(out=mv, in_=st)
                rs = stat.tile([P, 1], F32, tag="rs")
                nc.scalar.activation(rs, mv[:, 1:2], AF.Sqrt, bias=eps_t[:, :], scale=1.0)
                nc.vector.reciprocal(out=rs, in_=rs)
                ot = yout.tile([P, H], F32, tag="o")
                nc.vector.tensor_scalar(
                    out=ot, in0=yt, scalar1=mv[:, 0:1], scalar2=rs, op0=OP.subtract, op1=OP.mult
                )
                if not trivial_ln:
                    nc.vector.tensor_mul(out=ot, in0=ot, in1=gamma_t)
                    nc.vector.tensor_add(out=ot, in0=ot, in1=beta_t)
                nc.sync.dma_start(out=out[row, :], in_=ot)

    nc.compile()
    return nc


def _prepare(hidden_states, input_tensor, weight, bias, ln_gamma, ln_beta):
    B, S, Hdim = hidden_states.shape
    assert Hdim == H and B == N_CORES
    s_h = _scale_sym(hidden_states)
    s_w = _scale_sym(weight)
    deq = np.float32(1.0 / (np.float64(s_h) * np.float64(s_w)))

    bscaled = bias.astype(np.float64) * np.float64(s_h) * np.float64(s_w)
    b_hi = bscaled.astype(ml_dtypes.bfloat16)
    b_lo = (bscaled - b_hi.astype(np.float64)).astype(ml_dtypes.bfloat16)
    biasq = np.stack([b_hi, b_lo])  # [2, H] bf16

    trivial_ln = bool(np.all(ln_gamma == 1.0) and np.all(ln_beta == 0.0))

    ones2 = np.ones((2, P), dtype=ml_dtypes.bfloat16)
    ident = np.eye(P, dtype=ml_dtypes.bfloat16)

    common = {"w": np.ascontiguousarray(weight), "biasq": biasq, "ones2": ones2, "ident": ident}
    if not trivial_ln:
        common["gamma"] = np.ascontiguousarray(ln_gamma, dtype=np.float32)
        common["beta"] = np.ascontiguousarray(ln_beta, dtype=np.float32)

    in_maps = []
    for b in range(N_CORES):
        in_maps.append(
            {
                "hs": np.ascontiguousarray(hidden_states[b]),
                "res": np.ascontiguousarray(input_tensor[b]),
                **common,
            }
        )
    return s_h, s_w, deq, trivial_ln, in_maps, S


def _ensure_ntff_hook():
    """Provide antenv.axon_hooks if the image lacks it (NTFF tracing)."""
    import sys
    import types

    try:
        from antenv.axon_hooks import get_axon_ntff_profile_hook  # noqa: F401

        return
    except ImportError:
        pass
    from trn_agent_boot.trn_boot import _ntff_profile_via_ctypes

    hook = _ntff_profile_via_ctypes("/opt/axon/libaxon_pjrt.so")
    mod = types.ModuleType("antenv.axon_hooks")
    mod.get_axon_ntff_profile_hook = lambda: hook
    mod.set_axon_ntff_profile_hook = lambda h: None
    sys.modules["antenv.axon_hooks"] = mod


def run(hidden_states, input_tensor, weight, bias, ln_gamma, ln_beta, trace=False, **trace_kw):
    if trace:
        _ensure_ntff_hook()
    s_h, s_w, deq, trivial_ln, in_maps, S = _prepare(
        hidden_states, input_tensor, weight, bias, ln_gamma, ln_beta
    )
    nc = build_bass(S, s_h, s_w, deq, trivial_ln)
    kres = run_bass_kernel_spmd(nc, in_maps, list(range(N_CORES)), trace=trace, **trace_kw)
    out = np.stack([kres.results[i]["out"] for i in range(N_CORES)])
    return out, kres


def kernel(hidden_states, input_tensor, weight, bias, ln_gamma, ln_beta):
    out, _ = run(hidden_states, input_tensor, weight, bias, ln_gamma, ln_beta)
    return out


# revision 5
# speedup vs baseline: 1.0242x; 1.0242x over previous
"""Fused fake-quant GEMM + bias + residual + LayerNorm (BertSelfOutput) on 8 trn2 cores.

Strategy: data-parallel over the batch dim (B=8 -> one batch element per core).
Each core computes, for its [4096, 1024] shard:
    hq = fake_quant(hidden); wq = fake_quant(weight)
    h  = hq @ wq.T + bias;   y = h + input;   out = layernorm(y) * gamma + beta

Key trick: fake-quant values are integers in [-127, 127] after scaling, which
are exactly representable in bf16 -> the GEMM runs on the PE at full bf16 rate
with exact fp32 integer accumulation in PSUM; a single dequant multiply at the
end reproduces the reference to ~1e-6 relative.
"""

import numpy as np
import ml_dtypes

import concourse.bass as bass
import concourse.mybir as mybir
import concourse.tile as tile
from concourse import bacc
from concourse.bass_utils import run_bass_kernel_spmd

F32 = mybir.dt.float32
BF16 = mybir.dt.bfloat16
AF = mybir.ActivationFunctionType
OP = mybir.AluOpType

MAGIC = 12582912.0  # 1.5 * 2**23: (x + MAGIC) - MAGIC == rint(x) for |x| < 2**22
QMAX = 127.0
CLIP_VAL = 2.5
LN_EPS = 1e-12
H = 1024
N_CORES = 8
P = 128


def _scale_sym(x: np.ndarray) -> np.float32:
    """fp32-exact replica of the reference's per-tensor scale computation."""
    amax = np.float32(min(np.float32(np.abs(x).max()), np.float32(CLIP_VAL)))
    return np.float32(np.float32(QMAX) / np.maximum(amax, np.float32(1e-8)))


def build_bass(n_rows: int, s_h: float, s_w: float, deq: float, trivial_ln: bool):
    nc = bacc.Bacc(num_devices=N_CORES)
    KT = H // P  # 8 k-tiles
    MT = n_rows // P  # m-tiles per core

    hs = nc.declare_dram_parameter("hs", [n_rows, H], F32, isOutput=False)
    res = nc.declare_dram_parameter("res", [n_rows, H], F32, isOutput=False)
    w = nc.declare_dram_parameter("w", [H, H], F32, isOutput=False)
    biasq = nc.declare_dram_parameter("biasq", [2, H], BF16, isOutput=False)
    ones2 = nc.declare_dram_parameter("ones2", [2, P], BF16, isOutput=False)
    ident = nc.declare_dram_parameter("ident", [P, P], BF16, isOutput=False)
    if not trivial_ln:
        gamma = nc.declare_dram_parameter("gamma", [H], F32, isOutput=False)
        beta = nc.declare_dram_parameter("beta", [H], F32, isOutput=False)
    out = nc.declare_dram_parameter("out", [n_rows, H], F32, isOutput=True)

    with tile.TileContext(nc) as tc:
        with (
            tc.tile_pool(name="singles", bufs=1) as singles,
            tc.tile_pool(name="wprep", bufs=2) as wprep,
            tc.tile_pool(name="hin", bufs=3) as hin,
            tc.tile_pool(name="quant", bufs=2) as quant,
            tc.tile_pool(name="qt", bufs=2) as qtp,
            tc.tile_pool(name="resin", bufs=3) as resin,
            tc.tile_pool(name="yout", bufs=3) as yout,
            tc.tile_pool(name="stat", bufs=4) as stat,
            tc.tile_pool(name="pst", bufs=2, space="PSUM") as pst,
            tc.tile_pool(name="pso", bufs=2, space="PSUM") as pso_pool,
        ):
            # ---- constants
            id_t = singles.tile([P, P], BF16)
            nc.sync.dma_start(out=id_t, in_=ident[:, :])
            ones_t = singles.tile([2, P], BF16)
            nc.sync.dma_start(out=ones_t, in_=ones2[:, :])
            biasq_t = singles.tile([2, H], BF16)
            nc.sync.dma_start(out=biasq_t, in_=biasq[:, :])
            eps_t = singles.tile([P, 1], F32)
            nc.vector.memset(eps_t, LN_EPS)
            if not trivial_ln:
                gamma_t = singles.tile([P, H], F32)
                nc.sync.dma_start(
                    out=gamma_t,
                    in_=bass.AP(tensor=gamma.tensor, offset=0, ap=[[0, P], [1, H]]),
                )
                beta_t = singles.tile([P, H], F32)
                nc.sync.dma_start(
                    out=beta_t,
                    in_=bass.AP(tensor=beta.tensor, offset=0, ap=[[0, P], [1, H]]),
                )

            # ---- weight: quantize to integer-valued bf16, transpose to [h, o]
            # wqt[:, k, o] holds wq.T for k-tile k: partition = h within tile k.
            wqt = singles.tile([P, KT, H], BF16)
            for wb in range(KT):  # row-tiles of weight: [128 o, 1024 h]
                wt = wprep.tile([P, H], F32, tag="wt")
                nc.sync.dma_start(out=wt, in_=w[wb * P : (wb + 1) * P, :])
                # exact IEEE fp32 pipeline: clamp(x*s, +-127) then round-half-even
                aw = wprep.tile([P, H], F32, tag="aw")
                nc.vector.tensor_scalar(
                    out=aw, in0=wt, scalar1=float(s_w), scalar2=QMAX, op0=OP.mult, op1=OP.min
                )
                bw = wprep.tile([P, H], F32, tag="bw")
                nc.vector.tensor_scalar(
                    out=bw, in0=aw, scalar1=-QMAX, scalar2=MAGIC, op0=OP.max, op1=OP.add
                )
                qw = wprep.tile([P, H], BF16, tag="qw")
                nc.vector.tensor_scalar(
                    out=qw, in0=bw, scalar1=MAGIC, scalar2=None, op0=OP.subtract
                )
                pt = pst.tile([P, H], F32, tag="pt")
                for k in range(KT):
                    # psum[:, k*128:+128] = qw[:, k*128:+128].T  ([128 h, 128 o])
                    nc.tensor.matmul(
                        pt[:, k * P : (k + 1) * P],
                        lhsT=qw[:, k * P : (k + 1) * P],
                        rhs=id_t[:, :],
                        start=True,
                        stop=True,
                    )
                # copy transposed blocks into wqt[:, :, wb*128:+128] (cast to bf16)
                nc.scalar.activation(
                    wqt[:, :, wb * P : (wb + 1) * P],
                    pt.rearrange("p (k o) -> p k o", k=KT),
                    AF.Copy,
                    bias=0.0,
                    scale=1.0,
                )

            # ---- main loop over m-tiles
            for m in range(MT):
                row = slice(m * P, (m + 1) * P)
                ht = hin.tile([P, H], F32)
                nc.sync.dma_start(out=ht, in_=hs[row, :])
                # exact IEEE fp32: clamp(x*s_h, +-127), then round-half-even, cast bf16
                at = quant.tile([P, H], F32, tag="a")
                nc.vector.tensor_scalar(
                    out=at, in0=ht, scalar1=float(s_h), scalar2=QMAX, op0=OP.mult, op1=OP.min
                )
                bt = quant.tile([P, H], F32, tag="b")
                nc.vector.tensor_scalar(
                    out=bt, in0=at, scalar1=-QMAX, scalar2=MAGIC, op0=OP.max, op1=OP.add
                )
                qt = quant.tile([P, H], BF16, tag="q")
                nc.vector.tensor_scalar(
                    out=qt, in0=bt, scalar1=MAGIC, scalar2=None, op0=OP.subtract
                )
                # transpose q into [h, m] layout via PE
                ptt = pst.tile([P, H], F32, tag="pt")
                for k in range(KT):
                    nc.tensor.matmul(
                        ptt[:, k * P : (k + 1) * P],
                        lhsT=qt[:, k * P : (k + 1) * P],
                        rhs=id_t[:, :],
                        start=True,
                        stop=True,
                    )
                qtt = qtp.tile([P, H], BF16)
                nc.scalar.activation(qtt, ptt, AF.Copy, bias=0.0, scale=1.0)

                # main GEMM: pso[m, o] = sum_k qtt[:, k].T @ wqt[:, k] (+ bias rows)
                pso = pso_pool.tile([P, H], F32)
                for nh in range(2):
                    col = slice(nh * 512, (nh + 1) * 512)
                    for k in range(KT):
                        nc.tensor.matmul(
                            pso[:, col],
                            lhsT=qtt[:, k * P : (k + 1) * P],
                            rhs=wqt[:, k, col],
                            start=(k == 0),
                            stop=False,
                        )
                    nc.tensor.matmul(
                        pso[:, col],
                        lhsT=ones_t[:, :],
                        rhs=biasq_t[:, col],
                        start=False,
                        stop=True,
                    )

                rt_in = resin.tile([P, H], F32)
                nc.sync.dma_start(out=rt_in, in_=res[row, :])
                # y = pso * deq + input  (dequant + residual fused)
                yt = yout.tile([P, H], F32, tag="y")
                nc.vector.scalar_tensor_tensor(
                    out=yt, in0=pso, scalar=float(deq), in1=rt_in, op0=OP.mult, op1=OP.add
                )
                # layernorm stats
                st = stat.tile([P, 2, 6], F32, tag="st")
                for c in range(2):
                    nc.vector.bn_stats(out=st[:, c, :], in_=yt[:, c * 512 : (c + 1) * 512])
                mv = stat.tile([P, 2], F32, tag="mv")
                nc.vector.bn_aggr(out=mv, in_=st)
                rs = stat.tile([P, 1], F32, tag="rs")
                nc.scalar.activation(rs, mv[:, 1:2], AF.Sqrt, bias=eps_t[:, :], scale=1.0)
                nc.vector.reciprocal(out=rs, in_=rs)
                ot = yout.tile([P, H], F32, tag="o")
                nc.vector.tensor_scalar(
                    out=ot, in0=yt, scalar1=mv[:, 0:1], scalar2=rs, op0=OP.subtract, op1=OP.mult
                )
                if not trivial_ln:
                    nc.vector.tensor_mul(out=ot, in0=ot, in1=gamma_t)
                    nc.vector.tensor_add(out=ot, in0=ot, in1=beta_t)
                nc.sync.dma_start(out=out[row, :], in_=ot)

    nc.compile()
    return nc


def _prepare(hidden_states, input_tensor, weight, bias, ln_gamma, ln_beta):
    B, S, Hdim = hidden_states.shape
    assert Hdim == H and B == N_CORES
    s_h = _scale_sym(hidden_states)
    s_w = _scale_sym(weight)
    deq = np.float32(1.0 / (np.float64(s_h) * np.float64(s_w)))

    bscaled = bias.astype(np.float64) * np.float64(s_h) * np.float64(s_w)
    b_hi = bscaled.astype(ml_dtypes.bfloat16)
    b_lo = (bscaled - b_hi.astype(np.float64)).astype(ml_dtypes.bfloat16)
    biasq = np.stack([b_hi, b_lo])  # [2, H] bf16

    trivial_ln = bool(np.all(ln_gamma == 1.0) and np.all(ln_beta == 0.0))

    ones2 = np.ones((2, P), dtype=ml_dtypes.bfloat16)
    ident = np.eye(P, dtype=ml_dtypes.bfloat16)

    common = {"w": np.ascontiguousarray(weight), "biasq": biasq, "ones2": ones2, "ident": ident}
    if not trivial_ln:
        common["gamma"] = np.ascontiguousarray(ln_gamma, dtype=np.float32)
        common["beta"] = np.ascontiguousarray(ln_beta, dtype=np.float32)

    in_maps = []
    for b in range(N_CORES):
        in_maps.append(
            {
                "hs": np.ascontiguousarray(hidden_states[b]),
                "res": np.ascontiguousarray(input_tensor[b]),
                **common,
            }
        )
    return s_h, s_w, deq, trivial_ln, in_maps, S


def _ensure_ntff_hook():
    """Provide antenv.axon_hooks if the image lacks it (NTFF tracing)."""
    import sys
    import types

    try:
        from antenv.axon_hooks import get_axon_ntff_profile_hook  # noqa: F401

        return
    except ImportError:
        pass
    from trn_agent_boot.trn_boot import _ntff_profile_via_ctypes

    hook = _ntff_profile_via_ctypes("/opt/axon/libaxon_pjrt.so")
    mod = types.ModuleType("antenv.axon_hooks")
    mod.get_axon_ntff_profile_hook = lambda: hook
    mod.set_axon_ntff_profile_hook = lambda h: None
    sys.modules["antenv.axon_hooks"] = mod


def run(hidden_states, input_tensor, weight, bias, ln_gamma, ln_beta, trace=False, **trace_kw):
    if trace:
        _ensure_ntff_hook()
    s_h, s_w, deq, trivial_ln, in_maps, S = _prepare(
        hidden_states, input_tensor, weight, bias, ln_gamma, ln_beta
    )
    nc = build_bass(S, s_h, s_w, deq, trivial_ln)
    kres = run_bass_kernel_spmd(nc, in_maps, list(range(N_CORES)), trace=trace, **trace_kw)
    out = np.stack([kres.results[i]["out"] for i in range(N_CORES)])
    return out, kres


def kernel(hidden_states, input_tensor, weight, bias, ln_gamma, ln_beta):
    out, _ = run(hidden_states, input_tensor, weight, bias, ln_gamma, ln_beta)
    return out
